# revision 41
# baseline (speedup 1.0000x reference)
"""Trainium2 Bass kernel for NoSharingGraphConv.

out[b,w,m] = sum_{h,n} x[b,h,n] * adj[h,w] * W[h,w,n,m] + bias[m]
  B=4096, N=17 (graph nodes), FIN=FOUT=256.

Sharding (8 NeuronCores): 4 batch groups x 2 out-feature halves.
Core c handles batch rows [bg*1024, (bg+1)*1024) and out features
[mh*128, (mh+1)*128), bg = c>>1, mh = c&1: 1156 matmuls of
[128x128]x[128x512] per core, all engines' work perfectly balanced.

The kernel is PE-bound in steady state (one 512-col bf16 matmul every
~216ns = the 2.4GHz 128x128 array peak; fp8 e4m3 fails the 2e-2
accuracy gate so bf16 is the fastest usable dtype), so all the
recoverable time is in the prologue. Levers applied:
  - x travels as float8e3 (e4m3 is too coarse; e3m4's 4 mantissa bits
    keep max rel err ~1.5e-2 < 2e-2) and is upcast fp8->bf16 on the
    otherwise idle Vector engine. This halves the batch-half-0 stream
    that gates the first matmul group. x is pre-scaled by 2 and W by
    0.5 on the host (exact powers of two) to dodge the e3m4 denormal
    floor.
  - adj is folded into W on the host: no DVE scaling stage, W slabs
    feed matmuls straight from DMA.
  - The first six w-groups (batch-half 0) run chunk-interleaved: one
    x chunk feeds six matmuls, cutting prologue x-bandwidth demand 6x
    (~127GB/s total, within what the slow-starting DMA rings deliver).
    Slabs 0-5 stream in fine h-rounds paired with the x c-chunks in
    consumption order, round-robin over the three DMA rings
    (sync/scalar/gpsimd, ~165GB/s each); batch-half-1 and later slabs
    queue behind phase-0 on their rings. Slabs 0-5 stay resident
    (wpool bufs=7) so their half-1 groups never re-load W.
  - 20 full-array warm-up matmuls (memset junk) run from engine start:
    the HAM clock gate ignores narrow matmuls, and a full-array stream
    releases 1.2->2.4GHz ~3.4us in, right as the real matmuls begin.
    Any multi-us PE idle re-throttles the clock (MID window), so the
    prologue is paced to keep the PE continuously busy.
  - Per (w, bh): 34 accumulating bf16 matmuls into one PSUM bank; ACT
    evacuates with the bias add; the final group evacuates in two
    256-col chunks with the out-DMA issued from the sync engine so the
    last ACT and DMA overlap.
  - xt_sb is split into one tile per batch half so half-1 upcast
    writes never false-serialize against half-0 matmul reads.
  - Device writes out_t [17, 128, 1024] (w, m', b); host permutes.
Known-fixed costs: ~6.5us engine init, ~4.3us teardown, and an
external ~432ns stall every 10.79us of PE time (~5us total).
"""

import sys

if "/opt/trn_rl_repo" not in sys.path:
    sys.path.insert(0, "/opt/trn_rl_repo")

import numpy as np

B, N, FIN, FOUT = 4096, 17, 256, 256
NC = 8
NBG = 4  # batch groups
BS = B // NBG  # 1024 batch rows per core
MH = FOUT // 2  # 128 out features per core
KCH = N * FIN // 128  # 34 contraction chunks of 128
NBH = BS // 512  # 2 batch halves (matmul free dim 512)

_CACHE = {}

# x batch-half-0 DMA chunks (c ranges), paired with slab h-rounds
_XH0_CHUNKS = [(0, 2), (2, 5), (5, 9), (9, 12), (12, 16), (16, 20), (20, 23), (23, 26), (26, 30), (30, KCH)]


def _build_module():
    import concourse.mybir as mybir
    import concourse.tile as tile
    from concourse import bacc

    f32 = mybir.dt.float32
    bf16 = mybir.dt.bfloat16
    f8 = mybir.dt.float8e3

    nc = bacc.Bacc("TRN2", target_bir_lowering=False)

    # host-prepared, partition-major, batch-half-major:
    #   xt8[bh, p, c, b'] = e3m4(2 * x[bh*512+b', h, 2p+kc]), c = 2h+kc
    xt_d = nc.dram_tensor("xt8", [NBH, 128, KCH, 512], f8, kind="ExternalInput")
    # host-swizzled, adj-folded, pre-halved:
    #   w_sw[w, p, h, kc, m'] = bf16(0.5 * W[h, w, 2p+kc, mh*128+m'] * adj[h, w])
    w_d = nc.dram_tensor("w_sw", [N, 128, N, 2, MH], bf16, kind="ExternalInput")
    b_d = nc.dram_tensor("b", [MH], f32, kind="ExternalInput")
    o_d = nc.dram_tensor("out_t", [N, MH, BS], f32, kind="ExternalOutput")

    with tile.TileContext(nc) as tc:
        with (
            tc.tile_pool(name="const", bufs=1) as const,
            tc.tile_pool(name="wslab", bufs=9) as wpool,
            tc.tile_pool(name="obuf", bufs=4) as opool,
            tc.tile_pool(name="psum", bufs=8, space="PSUM") as psum,
        ):
            # PE warm-up: HAM watches whole-array activity; these junk
            # matmuls span all 128 partitions and release the clock
            # gate while phase-0 DMA streams.
            warm_w = const.tile([128, 128], bf16)
            warm_x = const.tile([128, 512], bf16)
            nc.vector.memset(warm_w[:], 0.0)
            nc.vector.memset(warm_x[:], 0.0)
            ps6 = []
            for i in range(8):
                psb = psum.tile([128, 512], f32, tag="ps", name=f"psblk{i}")
                ps6.append(psb)
            for _ in range(22):
                nc.tensor.matmul(
                    ps6[0][:], lhsT=warm_w[:], rhs=warm_x[:], start=True, stop=True
                )

            # bias half on partitions: bias_sb[p, 0] = b[mh*128 + p]
            bias_sb = const.tile([128, 1], f32)
            nc.gpsimd.dma_start(bias_sb[:], b_d[:][:, None])

            # resident bf16 x^T [128, c, b], one tile per batch half so
            # half-1 upcast writes never serialize against half-0 reads
            xt_sb0 = const.tile([128, KCH, 512], bf16)
            xt_sb1 = const.tile([128, KCH, 512], bf16)
            xt_sb = [xt_sb0, xt_sb1]
            stage0 = const.tile([128, KCH, 512], f8)
            stage1 = const.tile([128, KCH, 512], f8)
            stage = [stage0, stage1]

            def xt_load(bh, c0, c1, eng):
                eng.dma_start(stage[bh][:, c0:c1, :], xt_d[bh, :, c0:c1, :])

            def xt_upcast(bh, c0, c1):
                # fp8 -> bf16 converting copy on the DVE (GpSimd's
                # software tensor ops are 20x too slow, and ACT copies
                # showed pathological serialization against the PE).
                nc.vector.tensor_scalar_mul(
                    xt_sb[bh][:, c0:c1, :], stage[bh][:, c0:c1, :], 1.0
                )

            def load_slab(w, eng, chunks=None):
                wt = wpool.tile([128, N, 2, MH], bf16, tag="wslab")
                if chunks is None:
                    eng.dma_start(
                        wt[:].rearrange("p h kc m -> p (h kc m)"),
                        w_d[w].rearrange("p h kc m -> p (h kc m)"),
                    )
                else:
                    for (h0, h1), e in chunks:
                        e.dma_start(
                            wt[:, h0:h1].rearrange("p h kc m -> p (h kc m)"),
                            w_d[w, :, h0:h1].rearrange("p h kc m -> p (h kc m)"),
                        )
                return wt

            # phase-0: slabs 0..5 in h-chunks + x8 half0 c-chunks,
            # issued in consumption order round-robin over the three
            # DMA rings. The first six w-groups run chunk-interleaved,
            # so each x chunk feeds 6 matmuls: prologue DMA demand
            # drops to ~125GB/s and the PE never starves.
            _rr = [nc.sync, nc.scalar, nc.gpsimd]
            _ri = [0]

            def rr():
                e = _rr[_ri[0] % 3]
                _ri[0] += 1
                return e

            NW = 8  # interleaved prologue groups
            slabs = {}
            slab_chunks = {w: [] for w in range(NW)}
            for w in range(NW):
                slabs[w] = wpool.tile(
                    [128, N, 2, MH], bf16, tag="wslab", name=f"wt{w}"
                )
            # h-rounds paired with the x c-chunks consumed at those h's
            rounds = [
                ((0, 1), [(0, 2)]),
                ((1, 2), [(2, 5)]),
                ((2, 4), [(5, 9)]),
                ((4, 6), [(9, 12)]),
                ((6, 8), [(12, 16)]),
                ((8, 10), [(16, 20)]),
                ((10, 12), [(20, 23)]),
                ((12, 14), [(23, 26), (26, 30)]),
                ((14, N), [(30, KCH)]),
            ]
            for (h0, h1), xcs in rounds:
                xq = list(xcs)
                for w in range(NW):
                    rr().dma_start(
                        slabs[w][:, h0:h1].rearrange("p h kc m -> p (h kc m)"),
                        w_d[w, :, h0:h1].rearrange("p h kc m -> p (h kc m)"),
                    )
                    if w % 2 == 1 and xq:
                        c0, c1 = xq.pop(0)
                        xt_load(0, c0, c1, rr())
                        xt_upcast(0, c0, c1)

            def mm_group(wt, w, bh, split_evac=1, out_eng=None):
                ps = psum.tile([128, 512], mybir.dt.float32, tag="ps")
                for c in range(KCH):
                    h, kc = divmod(c, 2)
                    nc.tensor.matmul(
                        ps[:],
                        lhsT=wt[:, h, kc, :],
                        rhs=xt_sb[bh][:, c, :],
                        start=(c == 0),
                        stop=(c == KCH - 1),
                    )
                ot = opool.tile([128, 512], f32, tag="ot")
                step = 512 // split_evac
                for s in range(split_evac):
                    sl = slice(s * step, (s + 1) * step)
                    nc.scalar.activation(
                        ot[:, sl],
                        ps[:, sl],
                        mybir.ActivationFunctionType.Identity,
                        bias=bias_sb[:, 0:1],
                    )
                    (out_eng or nc.scalar).dma_start(
                        o_d[w, :, bh * 512 + s * step : bh * 512 + (s + 1) * step],
                        ot[:, sl],
                    )

            # interleaved b0 block: each x chunk feeds all six groups
            for c in range(KCH):
                h, kc = divmod(c, 2)
                for w in range(NW):
                    nc.tensor.matmul(
                        ps6[w][:],
                        lhsT=slabs[w][:, h, kc, :],
                        rhs=xt_sb[0][:, c, :],
                        start=(c == 0),
                        stop=(c == KCH - 1),
                    )
            for w in range(NW):
                ot = opool.tile([128, 512], f32, tag="ot")
                nc.scalar.activation(
                    ot[:],
                    ps6[w][:],
                    mybir.ActivationFunctionType.Identity,
                    bias=bias_sb[:, 0:1],
                )
                nc.scalar.dma_start(o_d[w, :, 0:512], ot[:])

            # near-term slab, then x half1 behind it per ring
            slabs[NW] = load_slab(NW, nc.gpsimd)
            for (c0, c1), eng in (
                ((0, 9), nc.sync),
                ((9, 17), nc.scalar),
                ((17, 26), nc.sync),
                ((26, KCH), nc.scalar),
            ):
                xt_load(1, c0, c1, eng)
                xt_upcast(1, c0, c1)

            for w in range(0, NW):
                mm_group(slabs[w], w, 1)
            for w in range(NW, N):
                if w + 1 < N:
                    slabs[w + 1] = load_slab(
                        w + 1,
                        None,
                        chunks=[((0, 9), nc.gpsimd), ((9, N), nc.sync)],
                    )
                mm_group(slabs[w], w, 0)
                if w == N - 1:
                    # final group as two 256-col accumulation sub-groups:
                    # the first half's evac+DMA overlap the second
                    # half's matmuls, shrinking the post-last-MM tail
                    ps = psum.tile([128, 512], mybir.dt.float32, tag="ps")
                    for s in range(2):
                        cols = slice(s * 256, (s + 1) * 256)
                        xcols = slice(512 + s * 256, 512 + (s + 1) * 256)
                        for c in range(KCH):
                            h, kc = divmod(c, 2)
                            nc.tensor.matmul(
                                ps[:, cols],
                                lhsT=slabs[w][:, h, kc, :],
                                rhs=xt_sb[1][:, c, s * 256 : (s + 1) * 256],
                                start=(c == 0),
                                stop=(c == KCH - 1),
                            )
                        ot = opool.tile([128, 512], f32, tag="ot", name=f"otf{s}")
                        nc.scalar.activation(
                            ot[:, cols],
                            ps[:, cols],
                            mybir.ActivationFunctionType.Identity,
                            bias=bias_sb[:, 0:1],
                        )
                        (nc.sync if s == 1 else nc.scalar).dma_start(
                            o_d[w, :, xcols], ot[:, cols]
                        )
                else:
                    mm_group(slabs[w], w, 1)

    nc.compile()
    return nc


def _get_module():
    if "nc" not in _CACHE:
        _CACHE["nc"] = _build_module()
    return _CACHE["nc"]


def kernel(x, adj, W, b, _trace=False):
    from concourse.bass_utils import run_bass_kernel_spmd

    x = np.ascontiguousarray(np.asarray(x, dtype=np.float32))
    adj = np.ascontiguousarray(np.asarray(adj, dtype=np.float32))
    W = np.ascontiguousarray(np.asarray(W, dtype=np.float32))
    b = np.ascontiguousarray(np.asarray(b, dtype=np.float32))

    nc = _get_module()

    import ml_dtypes

    # adj folded into W and pre-halved (compensates the 2x on x):
    #   [w, p, h, kc, m'] = 0.5 * (W * adj)[h, w, 2p+kc, mh*128+m']
    Wa = (0.5 * W) * adj[:, :, None, None]
    w_sw = []
    for mh in range(2):
        wh = Wa[:, :, :, mh * MH : (mh + 1) * MH]  # [h, w, n, m']
        wr = wh.reshape(N, N, FIN // 2, 2, MH)  # (h, w, p, kc, m')
        w_sw.append(
            np.ascontiguousarray(
                wr.transpose(1, 2, 0, 3, 4).astype(ml_dtypes.bfloat16)
            )
        )

    xt_by_bg = []
    for bg in range(NBG):
        xs = x[bg * BS : (bg + 1) * BS]  # [BS, N, FIN]
        # xt8[bh, p, c, b'] = e3m4(2 * x[bh*512+b', h, 2p+kc]), c = 2h+kc
        xr = (2.0 * xs).reshape(NBH, 512, N, FIN // 2, 2)  # (bh, b', h, p, kc)
        xt_by_bg.append(
            np.ascontiguousarray(
                xr.transpose(0, 3, 2, 4, 1)  # (bh, p, h, kc, b')
                .reshape(NBH, 128, KCH, 512)
                .astype(ml_dtypes.float8_e3m4)
            )
        )

    in_maps = []
    for c in range(NC):
        bg, mh = divmod(c, 2)
        in_maps.append(
            {
                "xt8": xt_by_bg[bg],
                "w_sw": w_sw[mh],
                "b": b[mh * MH : (mh + 1) * MH].copy(),
            }
        )

    res = run_bass_kernel_spmd(nc, in_maps, list(range(NC)), trace=_trace)
    _CACHE["last_result"] = res

    out = np.empty((B, N, FOUT), dtype=np.float32)
    for c in range(NC):
        bg, mh = divmod(c, 2)
        ot = res.results[c]["out_t"]  # [17, 128, 1024] = (w, m', b)
        out[bg * BS : (bg + 1) * BS, :, mh * MH : (mh + 1) * MH] = ot.transpose(
            2, 0, 1
        )
    return out


# revision 43
# speedup vs baseline: 1.0069x; 1.0069x over previous
"""Trainium2 Bass kernel for NoSharingGraphConv.

out[b,w,m] = sum_{h,n} x[b,h,n] * adj[h,w] * W[h,w,n,m] + bias[m]
  B=4096, N=17 (graph nodes), FIN=FOUT=256.

Sharding (8 NeuronCores): 4 batch groups x 2 out-feature halves.
Core c handles batch rows [bg*1024, (bg+1)*1024) and out features
[mh*128, (mh+1)*128), bg = c>>1, mh = c&1: 1156 matmuls of
[128x128]x[128x512] per core, all engines' work perfectly balanced.

v3 — the kernel is PE-bound in steady state (one 512-col bf16 matmul
every 216ns ~= the 2.4GHz array peak), so the remaining time is in the
prologue. Three levers applied:
  - x travels as float8e3 (e4m4 is too coarse; e3m4's 4 mantissa bits
    keep max rel err ~1.3e-2) and is upcast fp8->bf16 on the otherwise
    idle Vector engine. This halves the 4.5MB batch-half-0 stream that
    gates the first matmul group. x is pre-scaled by 2 and W by 0.5 on
    the host (exact power-of-2) to dodge the e3m4 denormal floor.
  - adj is folded into W on the host: no DVE scaling stage, W slabs
    feed matmuls straight from DMA.
  - Phase-0 traffic (slab0 + slab1 + x8 half0, 4.45MB) is spread
    across all three DMA rings (sync/scalar/gpsimd, ~165GB/s each) in
    consumption order; slabs 0/1 stream in h-chunks so group 0/1
    matmuls start before the full slab lands. Batch-half-1 data and
    later slabs queue behind phase-0 on their rings.
  - 16 full-array warm-up matmuls (memset junk) run from engine start:
    the HAM clock gate ignores narrow matmuls, and a full-array stream
    releases 1.2->2.4GHz ~3.4us in, right as the real matmuls begin.
    Group order runs all of batch-half-0 for w=0..5 first (slabs 0-5
    stay resident, wpool bufs=7) so batch-half-1 is never on the
    critical path.
  - Per (w, bh): 34 accumulating bf16 matmuls into one PSUM bank; ACT
    evacuates with the bias add; the final group evacuates in two
    256-col chunks so the last out-DMA overlaps the last ACT.
  - Device writes out_t [17, 128, 1024] (w, m', b); host permutes.
"""

import sys

if "/opt/trn_rl_repo" not in sys.path:
    sys.path.insert(0, "/opt/trn_rl_repo")

import numpy as np

B, N, FIN, FOUT = 4096, 17, 256, 256
NC = 8
NBG = 4  # batch groups
BS = B // NBG  # 1024 batch rows per core
MH = FOUT // 2  # 128 out features per core
KCH = N * FIN // 128  # 34 contraction chunks of 128
NBH = BS // 512  # 2 batch halves (matmul free dim 512)

_CACHE = {}

# x batch-half-0 DMA chunks (c ranges), paired with slab h-rounds
_XH0_CHUNKS = [(0, 2), (2, 5), (5, 9), (9, 12), (12, 16), (16, 20), (20, 23), (23, 26), (26, 30), (30, KCH)]


def _build_module():
    import concourse.mybir as mybir
    import concourse.tile as tile
    from concourse import bacc

    f32 = mybir.dt.float32
    bf16 = mybir.dt.bfloat16
    f8 = mybir.dt.float8e3

    nc = bacc.Bacc("TRN2", target_bir_lowering=False)

    # host-prepared, partition-major, batch-half-major:
    #   xt8[bh, p, c, b'] = e3m4(2 * x[bh*512+b', h, 2p+kc]), c = 2h+kc
    xt_d = nc.dram_tensor("xt8", [NBH, 128, KCH, 512], f8, kind="ExternalInput")
    # host-swizzled, adj-folded, pre-halved:
    #   w_sw[w, p, h, kc, m'] = bf16(0.5 * W[h, w, 2p+kc, mh*128+m'] * adj[h, w])
    w_d = nc.dram_tensor("w_sw", [N, 128, N, 2, MH], bf16, kind="ExternalInput")
    b_d = nc.dram_tensor("b", [MH], f32, kind="ExternalInput")
    o_d = nc.dram_tensor("out_t", [N, MH, BS], f32, kind="ExternalOutput")

    with tile.TileContext(nc) as tc:
        with (
            tc.tile_pool(name="const", bufs=1) as const,
            tc.tile_pool(name="wslab", bufs=7) as wpool,
            tc.tile_pool(name="obuf", bufs=4) as opool,
            tc.tile_pool(name="psum", bufs=7, space="PSUM") as psum,
        ):
            # PE warm-up: HAM watches whole-array activity; these junk
            # matmuls span all 128 partitions and release the clock
            # gate while phase-0 DMA streams.
            warm_w = const.tile([128, 128], bf16)
            warm_x = const.tile([128, 512], bf16)
            nc.vector.memset(warm_w[:], 0.0)
            nc.vector.memset(warm_x[:], 0.0)
            warm_ps = psum.tile([128, 512], f32, tag="ps")
            for _ in range(20):
                nc.tensor.matmul(
                    warm_ps[:], lhsT=warm_w[:], rhs=warm_x[:], start=True, stop=True
                )

            # bias half on partitions: bias_sb[p, 0] = b[mh*128 + p]
            bias_sb = const.tile([128, 1], f32)
            nc.gpsimd.dma_start(bias_sb[:], b_d[:][:, None])

            # resident bf16 x^T [128, c, b], one tile per batch half so
            # half-1 upcast writes never serialize against half-0 reads
            xt_sb0 = const.tile([128, KCH, 512], bf16)
            xt_sb1 = const.tile([128, KCH, 512], bf16)
            xt_sb = [xt_sb0, xt_sb1]
            stage0 = const.tile([128, KCH, 512], f8)
            stage1 = const.tile([128, KCH, 512], f8)
            stage = [stage0, stage1]

            def xt_load(bh, c0, c1, eng):
                eng.dma_start(stage[bh][:, c0:c1, :], xt_d[bh, :, c0:c1, :])

            def xt_upcast(bh, c0, c1):
                # fp8 -> bf16 converting copy on the DVE (GpSimd's
                # software tensor ops are 20x too slow, and ACT copies
                # showed pathological serialization against the PE).
                nc.vector.tensor_scalar_mul(
                    xt_sb[bh][:, c0:c1, :], stage[bh][:, c0:c1, :], 1.0
                )

            def load_slab(w, eng, chunks=None):
                wt = wpool.tile([128, N, 2, MH], bf16, tag="wslab")
                if chunks is None:
                    eng.dma_start(
                        wt[:].rearrange("p h kc m -> p (h kc m)"),
                        w_d[w].rearrange("p h kc m -> p (h kc m)"),
                    )
                else:
                    for (h0, h1), e in chunks:
                        e.dma_start(
                            wt[:, h0:h1].rearrange("p h kc m -> p (h kc m)"),
                            w_d[w, :, h0:h1].rearrange("p h kc m -> p (h kc m)"),
                        )
                return wt

            # phase-0: slabs 0..5 in h-chunks + x8 half0 c-chunks,
            # issued in consumption order round-robin over the three
            # DMA rings. The first six w-groups run chunk-interleaved,
            # so each x chunk feeds 6 matmuls: prologue DMA demand
            # drops to ~125GB/s and the PE never starves.
            _rr = [nc.sync, nc.scalar, nc.gpsimd]
            _ri = [0]

            def rr():
                e = _rr[_ri[0] % 3]
                _ri[0] += 1
                return e

            NW = 6  # interleaved prologue groups
            slabs = {}
            slab_chunks = {w: [] for w in range(NW)}
            for w in range(NW):
                slabs[w] = wpool.tile(
                    [128, N, 2, MH], bf16, tag="wslab", name=f"wt{w}"
                )
            # h-rounds paired with the x c-chunks consumed at those h's
            rounds = [
                ((0, 1), [(0, 1)]),
                ((1, 2), [(1, 3)]),
                ((2, 4), [(3, 6), (6, 9)]),
                ((4, 6), [(9, 12)]),
                ((6, 8), [(12, 16)]),
                ((8, 10), [(16, 20)]),
                ((10, 12), [(20, 23)]),
                ((12, 14), [(23, 26), (26, 30)]),
                ((14, 16), [(30, 32)]),
                ((16, N), [(32, KCH)]),
            ]
            for (h0, h1), xcs in rounds:
                xq = list(xcs)
                for w in range(NW):
                    rr().dma_start(
                        slabs[w][:, h0:h1].rearrange("p h kc m -> p (h kc m)"),
                        w_d[w, :, h0:h1].rearrange("p h kc m -> p (h kc m)"),
                    )
                    if w % 2 == 1 and xq:
                        c0, c1 = xq.pop(0)
                        xt_load(0, c0, c1, rr())
                        xt_upcast(0, c0, c1)

            def mm_group(wt, w, bh, split_evac=1, out_eng=None):
                ps = psum.tile([128, 512], mybir.dt.float32, tag="ps")
                for c in range(KCH):
                    h, kc = divmod(c, 2)
                    nc.tensor.matmul(
                        ps[:],
                        lhsT=wt[:, h, kc, :],
                        rhs=xt_sb[bh][:, c, :],
                        start=(c == 0),
                        stop=(c == KCH - 1),
                    )
                ot = opool.tile([128, 512], f32, tag="ot")
                step = 512 // split_evac
                for s in range(split_evac):
                    sl = slice(s * step, (s + 1) * step)
                    nc.scalar.activation(
                        ot[:, sl],
                        ps[:, sl],
                        mybir.ActivationFunctionType.Identity,
                        bias=bias_sb[:, 0:1],
                    )
                    (out_eng or nc.scalar).dma_start(
                        o_d[w, :, bh * 512 + s * step : bh * 512 + (s + 1) * step],
                        ot[:, sl],
                    )

            # interleaved b0 block: each x chunk feeds all six groups
            ps6 = []
            for w in range(NW):
                psw = psum.tile([128, 512], mybir.dt.float32, tag="ps")
                ps6.append(psw)
            for c in range(KCH):
                h, kc = divmod(c, 2)
                for w in range(NW):
                    nc.tensor.matmul(
                        ps6[w][:],
                        lhsT=slabs[w][:, h, kc, :],
                        rhs=xt_sb[0][:, c, :],
                        start=(c == 0),
                        stop=(c == KCH - 1),
                    )
            for w in range(NW):
                ot = opool.tile([128, 512], f32, tag="ot")
                nc.scalar.activation(
                    ot[:],
                    ps6[w][:],
                    mybir.ActivationFunctionType.Identity,
                    bias=bias_sb[:, 0:1],
                )
                nc.scalar.dma_start(o_d[w, :, 0:512], ot[:])

            # near-term slab, then x half1 behind it per ring
            slabs[NW] = load_slab(NW, nc.gpsimd)
            for (c0, c1), eng in (
                ((0, 9), nc.sync),
                ((9, 17), nc.scalar),
                ((17, 26), nc.sync),
                ((26, KCH), nc.scalar),
            ):
                xt_load(1, c0, c1, eng)
                xt_upcast(1, c0, c1)

            for w in range(0, NW):
                mm_group(slabs[w], w, 1)
            for w in range(NW, N):
                if w + 1 < N:
                    slabs[w + 1] = load_slab(w + 1, nc.gpsimd)
                mm_group(slabs[w], w, 0)
                if w == N - 1:
                    mm_group(slabs[w], w, 1, split_evac=2, out_eng=nc.sync)
                else:
                    mm_group(slabs[w], w, 1)

    nc.compile()
    return nc


def _get_module():
    if "nc" not in _CACHE:
        _CACHE["nc"] = _build_module()
    return _CACHE["nc"]


def kernel(x, adj, W, b, _trace=False):
    from concourse.bass_utils import run_bass_kernel_spmd

    x = np.ascontiguousarray(np.asarray(x, dtype=np.float32))
    adj = np.ascontiguousarray(np.asarray(adj, dtype=np.float32))
    W = np.ascontiguousarray(np.asarray(W, dtype=np.float32))
    b = np.ascontiguousarray(np.asarray(b, dtype=np.float32))

    nc = _get_module()

    import ml_dtypes

    # adj folded into W and pre-halved (compensates the 2x on x):
    #   [w, p, h, kc, m'] = 0.5 * (W * adj)[h, w, 2p+kc, mh*128+m']
    Wa = (0.5 * W) * adj[:, :, None, None]
    w_sw = []
    for mh in range(2):
        wh = Wa[:, :, :, mh * MH : (mh + 1) * MH]  # [h, w, n, m']
        wr = wh.reshape(N, N, FIN // 2, 2, MH)  # (h, w, p, kc, m')
        w_sw.append(
            np.ascontiguousarray(
                wr.transpose(1, 2, 0, 3, 4).astype(ml_dtypes.bfloat16)
            )
        )

    xt_by_bg = []
    for bg in range(NBG):
        xs = x[bg * BS : (bg + 1) * BS]  # [BS, N, FIN]
        # xt8[bh, p, c, b'] = e3m4(2 * x[bh*512+b', h, 2p+kc]), c = 2h+kc
        xr = (2.0 * xs).reshape(NBH, 512, N, FIN // 2, 2)  # (bh, b', h, p, kc)
        xt_by_bg.append(
            np.ascontiguousarray(
                xr.transpose(0, 3, 2, 4, 1)  # (bh, p, h, kc, b')
                .reshape(NBH, 128, KCH, 512)
                .astype(ml_dtypes.float8_e3m4)
            )
        )

    in_maps = []
    for c in range(NC):
        bg, mh = divmod(c, 2)
        in_maps.append(
            {
                "xt8": xt_by_bg[bg],
                "w_sw": w_sw[mh],
                "b": b[mh * MH : (mh + 1) * MH].copy(),
            }
        )

    res = run_bass_kernel_spmd(nc, in_maps, list(range(NC)), trace=_trace)
    _CACHE["last_result"] = res

    out = np.empty((B, N, FOUT), dtype=np.float32)
    for c in range(NC):
        bg, mh = divmod(c, 2)
        ot = res.results[c]["out_t"]  # [17, 128, 1024] = (w, m', b)
        out[bg * BS : (bg + 1) * BS, :, mh * MH : (mh + 1) * MH] = ot.transpose(
            2, 0, 1
        )
    return out
